# revision 24
# baseline (speedup 1.0000x reference)
"""FMM (dual-modality attention) Trainium2 kernel.

Problem: b=4, c=256, w=h=64 (n=4096), float32.
  K/Kth/V/Vth = 1x1-conv projections of x / x_th.
  sim  = softmax_m(Kth^T K)            [n, n] per sample
  x_1  = V @ sim * gamma + x           (contraction over softmax ROWS)
  E    = Vth V^T (full n), sim_c = softmax_d(max_d E - E)   [c, c]
  x_2  = gamma2 * sim_c @ V + x_1      (same sim_c for both modalities)

Sharding: 8 cores = (4 samples) x (2 halves of the n rows of sim).
Each core computes: S rows for its half (softmax over full m, float32r
logit path), partial P = V[:,half] @ exp_sim (bf16, summed on host),
full channel attention for its output columns (duplicated per pair).
Host epilogue: concat column-sharded A = gamma2*x2att + x, and add the
row-partial P contributions (gamma already applied on device).
"""

import os
import sys

sys.path.insert(0, "/opt/trn_rl_repo")

_ABLATE = set(os.environ.get("BASS_ABLATE", "").split(","))

import numpy as np
import ml_dtypes

import concourse.bass as bass
import concourse.bacc as bacc
import concourse.tile as tile
from concourse import mybir
from concourse.bass_utils import run_bass_kernel_spmd

F32 = mybir.dt.float32
F32R = mybir.dt.float32r
BF16 = mybir.dt.bfloat16
FP8 = mybir.dt.float8e4   # ml_dtypes.float8_e4m3: max 240, min normal 2^-6
LN2 = 0.6931471805599453
K_EXP = 7     # esim scaled so per-row max = 2^7 = 128 (fp8-safe)
VTS_SH = 10   # vts = V * rinv * 2^10; host folds 2^-10 into gamma

C = 256          # channels
N = 4096         # w*h
HALF = N // 2    # rows per core
P = 128          # partitions
CH = C // P      # channel halves (2)
MC = 512         # m chunk (one PSUM bank of f32)
NMC = N // MC    # 8 chunks over full m
NMC_H = HALF // MC   # 4 chunks over half
NBLK = N // P    # 32 n-blocks of 128
HBLK = HALF // P  # 16 row-blocks per core
SUPER = 2        # super-iterations over row-blocks
BPS = HBLK // SUPER  # row-blocks per super-iteration (4)
WCOLS = 2048 + 8 + 2 * C  # packed const buffer columns


def _f32r(ap):
    return ap.bitcast(F32R)


def build_program(g_rgb, g_th, g2_rgb, g2_th):
    nc = bacc.Bacc("TRN2", target_bir_lowering=False, debug=False, num_devices=8)

    def din(name, shape):
        return nc.dram_tensor(name, shape, F32, kind="ExternalInput").ap()

    x_d = nc.dram_tensor("x", [C, N], F32R, kind="ExternalInput").ap()
    xthmy_r_d = nc.dram_tensor("xth_my_r", [C, HALF], F32R,
                               kind="ExternalInput").ap()
    xv_d = din("xv", [C, N])         # reordered: my half first (V^T path)
    xvth_d = din("xvth", [C, N])     # reordered x_th (Vth^T path)
    xmy_d = din("x_my", [C, HALF])   # my natural columns of x
    xthmy_d = din("xth_my", [C, HALF])
    wconst_d = din("wconst", [P, WCOLS])  # packed weights/biases (one DMA)

    P_d = nc.dram_tensor("P_out", [SUPER, C, N], BF16, kind="ExternalOutput").ap()
    Pth_d = nc.dram_tensor("Pth_out", [SUPER, C, N], BF16, kind="ExternalOutput").ap()
    A_d = nc.dram_tensor("A_out", [C, HALF], F32, kind="ExternalOutput").ap()
    Ath_d = nc.dram_tensor("Ath_out", [C, HALF], F32, kind="ExternalOutput").ap()

    with tile.TileContext(nc) as tc:
        _build_tile(tc, nc,
                    x_d, xthmy_r_d, xv_d, xvth_d, xmy_d, xthmy_d, wconst_d,
                    P_d, Pth_d, A_d, Ath_d,
                    g_rgb, g_th, g2_rgb, g2_th)
    nc.finalize()
    return nc


def _build_tile(tc, nc, x_d, xthmy_r_d, xv_d, xvth_d, xmy_d, xthmy_d,
                wconst_d, P_d, Pth_d, A_d, Ath_d, g_rgb, g_th, g2_rgb, g2_th):
    from contextlib import ExitStack

    Ident = mybir.ActivationFunctionType.Identity
    Exp = mybir.ActivationFunctionType.Exp
    Alu = mybir.AluOpType
    AX = mybir.AxisListType

    scratch_d = nc.dram_tensor("warm_scratch", [P, 64], F32).ap()
    scratch_bf_d = nc.dram_tensor("warm_scratch_bf", [P, 16], BF16).ap()

    ctx = ExitStack()
    with ctx:
        const = ctx.enter_context(tc.tile_pool(name="const", bufs=1))

        # ---- weights / biases: ONE DMA from the packed host buffer ----
        const_sb = const.tile([P, WCOLS], F32, tag="wconst")
        nc.sync.dma_start(out=const_sb[:, 0:1024], in_=wconst_d[:, 0:1024])
        nc.sync.dma_start(out=const_sb[:, 1024:2048], in_=wconst_d[:, 1024:2048])
        nc.sync.dma_start(out=const_sb[:, 2048:], in_=wconst_d[:, 2048:])

        # fp32r rounding copy for the logit-path weights (wkT | wkthT)
        wk_r = const.tile([P, 1024], F32R, tag="wk_r")
        nc.vector.tensor_copy(out=wk_r[:], in_=const_sb[:, 0:1024])
        # bf16 cast for the value-path weights (wvT | wvthT)
        wv_bf = const.tile([P, 1024], BF16, tag="wv_bf")
        nc.vector.tensor_copy(out=wv_bf[:], in_=const_sb[:, 1024:2048])

        def wk_lhsT(ci, co):      # Wk_rgb^T slice [c_in 128, c_out 128]
            return wk_r[:, ci * C + co * P: ci * C + co * P + P]

        def wkth_lhsT(ci, co):
            return wk_r[:, 512 + ci * C + co * P: 512 + ci * C + co * P + P]

        def wv_lhsT(ci, co):
            return wv_bf[:, ci * C + co * P: ci * C + co * P + P]

        def wvth_lhsT(ci, co):
            return wv_bf[:, 512 + ci * C + co * P: 512 + ci * C + co * P + P]

        def wv_rhs(ci):           # [c_in 128, c_out 256] moving operand
            return wv_bf[:, ci * C: (ci + 1) * C]

        def wvth_rhs(ci):
            return wv_bf[:, 512 + ci * C: 512 + (ci + 1) * C]

        BOFF = 2048
        bk = const_sb[:, BOFF + 0: BOFF + 2]
        bkth = const_sb[:, BOFF + 2: BOFF + 4]
        bv = const_sb[:, BOFF + 4: BOFF + 6]
        bvth = const_sb[:, BOFF + 6: BOFF + 8]
        bvb = const_sb[:, BOFF + 8: BOFF + 8 + C]
        bvthb = const_sb[:, BOFF + 8 + C: BOFF + 8 + 2 * C]

        identity = const.tile([P, P], F32, tag="ident")
        from concourse.masks import make_identity
        make_identity(nc, identity[:])

        # ---- persistent projection outputs ----
        proj = ctx.enter_context(tc.tile_pool(name="proj", bufs=1))
        K_sb = proj.tile([P, CH, N], F32R, tag="K")          # 32KB/part
        Kth_sb = proj.tile([P, CH, HALF], F32R, tag="Kth")   # 16KB/part
        VT_sb = proj.tile([P, HBLK, C], BF16, tag="VT")      # 8KB/part (my half)
        VthT_sb = proj.tile([P, HBLK, C], BF16, tag="VthT")
        Vn_sb = proj.tile([P, CH, HALF], BF16, tag="Vn")     # 8KB/part
        Vthn_sb = proj.tile([P, CH, HALF], BF16, tag="Vthn")
        E_sb = proj.tile([P, CH, C], F32, tag="E")           # 2KB/part

        # energy PSUM accumulates across the whole V^T loop
        epsum_cm = tc.tile_pool(name="e_psum", bufs=2, space="PSUM", side="right")
        epsum = epsum_cm.__enter__()
        eps = []
        for _ch in range(CH):
            eps_t = epsum.tile([P, C], F32, tag="eps")
            eps.append(eps_t)

        # =============== phase 1: projections (+ interleaved energy) ======
        with ExitStack() as pctx:
            xfull = pctx.enter_context(tc.tile_pool(name="xfull", bufs=1))
            tr = pctx.enter_context(tc.tile_pool(name="proj_tr", bufs=3))
            ppsum = pctx.enter_context(tc.tile_pool(name="proj_psum", bufs=3, space="PSUM"))

            # xv / xvth pieces: issue the casting DMAs first so the V^T
            # loop never waits on them
            NQ = N // 4
            xv_pieces = {}
            for (nm, src_dd) in (("xv", xv_d), ("xvth", xvth_d)):
                for ci in range(CH):
                    for q in range(4):
                        piece = xfull.tile([P, NQ], BF16, tag=f"{nm}_{ci}_{q}")
                        nc.gpsimd.dma_start(
                            out=piece[:],
                            in_=src_dd[ci * P:(ci + 1) * P, q * NQ:(q + 1) * NQ])
                        xv_pieces[(nm, ci, q)] = piece

            # K natural [c_out, m] from x natural (f32r logit path), streamed;
            # f32 DMA chunks are bitcast to f32r (bit-identical) for the PE
            for mc in range(NMC):
                xc = tr.tile([P, CH, MC], F32R, tag="xk_c")
                for ci in range(CH):
                    nc.sync.dma_start(out=xc[:, ci, :],
                                      in_=x_d[ci * P:(ci + 1) * P, mc * MC:(mc + 1) * MC])
                for co in range(CH):
                    ps = ppsum.tile([P, MC], F32, tag="pp")
                    for ci in range(CH):
                        nc.tensor.matmul(
                            ps[:],
                            wk_lhsT(ci, co),
                            xc[:, ci, :],
                            start=(ci == 0), stop=(ci == CH - 1))
                    nc.scalar.activation(out=K_sb[:, co, mc * MC:(mc + 1) * MC],
                                         in_=ps[:], func=Ident,
                                         bias=bk[:, co:co + 1])

            # Kth [c_out, i(my rows)] from xth_my, streamed
            for mc in range(NMC_H):
                xc = tr.tile([P, CH, MC], F32R, tag="xk_c")
                for ci in range(CH):
                    nc.sync.dma_start(out=xc[:, ci, :],
                                      in_=xthmy_r_d[ci * P:(ci + 1) * P, mc * MC:(mc + 1) * MC])
                for co in range(CH):
                    ps = ppsum.tile([P, MC], F32, tag="pp")
                    for ci in range(CH):
                        nc.tensor.matmul(
                            ps[:],
                            wkth_lhsT(ci, co),
                            xc[:, ci, :],
                            start=(ci == 0), stop=(ci == CH - 1))
                    nc.scalar.activation(out=Kth_sb[:, co, mc * MC:(mc + 1) * MC],
                                         in_=ps[:], func=Ident,
                                         bias=bkth[:, co:co + 1])

            # V / Vth natural over my columns (bf16 value path), streamed
            for (src_d, wlhsT, bias, dst) in ((xmy_d, wv_lhsT, bv, Vn_sb),
                                              (xthmy_d, wvth_lhsT, bvth, Vthn_sb)):
                for mc in range(NMC_H):
                    xcb = tr.tile([P, CH, MC], BF16, tag="xmy_cb")
                    for ci in range(CH):
                        nc.gpsimd.dma_start(out=xcb[:, ci, :],
                                            in_=src_d[ci * P:(ci + 1) * P, mc * MC:(mc + 1) * MC])
                    for co in range(CH):
                        ps = ppsum.tile([P, MC], F32, tag="pp")
                        for ci in range(CH):
                            nc.tensor.matmul(
                                ps[:],
                                wlhsT(ci, co),
                                xcb[:, ci, :],
                                start=(ci == 0), stop=(ci == CH - 1))
                        nc.scalar.activation(out=dst[:, co, mc * MC:(mc + 1) * MC],
                                             in_=ps[:], func=Ident,
                                             bias=bias[:, co:co + 1])

            # V^T / Vth^T [n, c] from reordered xv / xvth, single full-tile
            # DMAs (keeps every downstream cone on 2 DMA queues), with the
            # energy matmuls interleaved per block
            BPQ = NQ // P  # blocks per quarter-piece (8)
            # other-half V^T blocks only feed the energy matmul; keep them
            # in the transient phase-1 pool
            VT_oth = xfull.tile([P, HBLK, C], BF16, tag="VT_oth")
            VthT_oth = xfull.tile([P, HBLK, C], BF16, tag="VthT_oth")
            for blk in range(NBLK):
                q, r = divmod(blk, BPQ)
                mine = blk < HBLK
                bi = blk if mine else blk - HBLK
                for (nm, wrhs, bbc, dst) in (
                        ("xv", wv_rhs, bvb, VT_sb if mine else VT_oth),
                        ("xvth", wvth_rhs, bvthb, VthT_sb if mine else VthT_oth)):
                    ps = ppsum.tile([P, C], F32, tag="ppv")
                    for ci in range(CH):
                        nc.tensor.matmul(
                            ps[:],
                            xv_pieces[(nm, ci, q)][:, r * P:(r + 1) * P],
                            wrhs(ci),
                            start=(ci == 0), stop=(ci == CH - 1))
                    nc.vector.tensor_add(dst[:, bi, :], ps[:], bbc[:])
                # energy: E[c, d] += Vth^T[blk]^T @ V^T[blk]
                vt_e = VT_sb if mine else VT_oth
                vtht_e = VthT_sb if mine else VthT_oth
                for ch in range(CH):
                    nc.tensor.matmul(eps[ch][:],
                                     vtht_e[:, bi, ch * P:(ch + 1) * P],
                                     vt_e[:, bi, :],
                                     start=(blk == 0), stop=(blk == NBLK - 1))

            # evict energy to SBUF now so phase 3 gets all 8 PSUM banks
            for ch in range(CH):
                nc.vector.tensor_copy(out=E_sb[:, ch, :], in_=eps[ch][:])
        epsum_cm.__exit__(None, None, None)

        # =============== phase 3: spatial attention ===============
        with ExitStack() as sctx:
            spool = sctx.enter_context(tc.tile_pool(name="srow", bufs=2))
            epool = sctx.enter_context(tc.tile_pool(name="expsim", bufs=BPS + 1))
            vtspool = sctx.enter_context(tc.tile_pool(name="vts", bufs=2 * BPS + 4))
            stat = sctx.enter_context(tc.tile_pool(name="stat", bufs=4))
            pstage = sctx.enter_context(tc.tile_pool(name="pstage", bufs=2))
            # (pstage tiles are half-row; two DMAs per (t, ch))
            spsum = sctx.enter_context(tc.tile_pool(name="s_psum", bufs=2, space="PSUM"))
            sbigp = sctx.enter_context(tc.tile_pool(name="sbig_psum", bufs=1, space="PSUM"))
            ppsum2 = sctx.enter_context(tc.tile_pool(name="p_psum", bufs=2, space="PSUM"))

            DR = mybir.MatmulPerfMode.DoubleRow
            NPAIR = BPS // 2
            NMM = 2 * NPAIR

            class PEmitter:
                """Emits the P = V @ exp_sim work of one super in small
                slices so it overlaps the next super's S phase."""

                def __init__(self, si, ep, vh, vl):
                    self.si = si
                    self.ep = ep
                    self.vh = vh
                    self.vl = vl
                    self.units = [(t, ch, mc) for t in range(2)
                                  for ch in range(CH) for mc in range(NMC)]
                    self.idx = 0
                    self.sts = {}

                def emit(self, k):
                    if "pmm" in _ABLATE:
                        self.idx = len(self.units)
                        return
                    for _ in range(k):
                        if self.idx >= len(self.units):
                            return
                        t, ch, mc = self.units[self.idx]
                        self.idx += 1
                        half, mco = divmod(mc, NMC // 2)
                        if mco == 0:
                            st_h = pstage.tile([P, N // 2], BF16, tag="pst",
                                               bufs=2)
                            self.sts[(t, ch, half)] = st_h
                        st = self.sts[(t, ch, half)]
                        ps = ppsum2.tile([P, MC], F32, tag="ppp")
                        for mi in range(NMM):
                            pi, hl = divmod(mi, 2)
                            vv = self.vh[(pi, t)] if hl == 0 else self.vl[(pi, t)]
                            nc.tensor.matmul(
                                ps[:],
                                vv[:, :, ch * P:(ch + 1) * P],
                                self.ep[pi][:, :, mc * MC:(mc + 1) * MC],
                                start=(mi == 0), stop=(mi == NMM - 1),
                                perf_mode=DR)
                        if mc % 2 == 0:
                            nc.scalar.copy(out=st[:, mco * MC:(mco + 1) * MC],
                                           in_=ps[:])
                        else:
                            nc.vector.tensor_copy(
                                out=st[:, mco * MC:(mco + 1) * MC], in_=ps[:])
                        if mco == NMC // 2 - 1:
                            out_d = P_d if t == 0 else Pth_d
                            nc.sync.dma_start(
                                out=out_d[self.si, ch * P:(ch + 1) * P,
                                          half * (N // 2):(half + 1) * (N // 2)],
                                in_=st[:])

            UNITS_PER_BLK = (2 * CH * NMC + BPS - 1) // BPS
            prev = None
            for si in range(SUPER):
                ep = []
                for _pi in range(NPAIR):
                    ep_t = epool.tile([P, 2, N], FP8, tag="esim", bufs=6)
                    ep.append(ep_t)
                vh = {}
                vl = {}
                for pi in range(NPAIR):
                    for t in range(2):
                        vh_t = vtspool.tile([P, 2, C], FP8, tag=f"vh{t}", bufs=9)
                        vl_t = vtspool.tile([P, 2, C], FP8, tag=f"vl{t}", bufs=9)
                        vh[(pi, t)] = vh_t
                        vl[(pi, t)] = vl_t
                for j in range(BPS):
                    pi, sub = divmod(j, 2)
                    blk = si * BPS + j
                    srow = spool.tile([P, N // 2], F32, tag="srow")
                    sbig = sbigp.tile([P, N // 2], F32, tag="sbig")
                    for mc in range(NMC):
                        lo = mc < NMC // 2
                        if lo:
                            ps_t = spsum.tile([P, MC], F32, tag="sps")
                            ps = ps_t[:]
                        else:
                            ps = sbig[:, (mc - NMC // 2) * MC:
                                      (mc - NMC // 2 + 1) * MC]
                        if "smm" not in _ABLATE:
                            for ci in range(CH):
                                nc.tensor.matmul(
                                    ps,
                                    Kth_sb[:, ci, blk * P:(blk + 1) * P],
                                    K_sb[:, ci, mc * MC:(mc + 1) * MC],
                                    start=(ci == 0), stop=(ci == CH - 1))
                        if lo:
                            # first half: evict to SBUF (alternate engines)
                            if mc % 2 == 0:
                                nc.scalar.copy(
                                    out=srow[:, mc * MC:(mc + 1) * MC],
                                    in_=ps)
                            else:
                                nc.vector.tensor_copy(
                                    out=srow[:, mc * MC:(mc + 1) * MC],
                                    in_=ps)
                        # second half: logits stay in PSUM (sbig)

                    red1 = stat.tile([P, 1], F32, tag="red1")
                    nc.vector.tensor_reduce(out=red1[:], in_=srow[:],
                                            axis=AX.X, op=Alu.max)
                    red2 = stat.tile([P, 1], F32, tag="red2")
                    nc.vector.tensor_reduce(out=red2[:], in_=sbig[:],
                                            axis=AX.X, op=Alu.max)
                    gmax = stat.tile([P, 1], F32, tag="gmax")
                    nc.vector.tensor_max(gmax[:], red1[:], red2[:])
                    nmax7 = stat.tile([P, 1], F32, tag="nmax7")
                    nc.vector.tensor_scalar(
                        out=nmax7[:], in0=gmax[:], scalar1=-1.0,
                        scalar2=K_EXP * LN2, op0=Alu.mult, op1=Alu.add)
                    rs1 = stat.tile([P, 1], F32, tag="rs1")
                    rs2 = stat.tile([P, 1], F32, tag="rs2")
                    nc.scalar.activation(out=ep[pi][:, sub, N // 2:N],
                                         in_=sbig[:], func=Exp, bias=nmax7[:],
                                         accum_out=rs2[:])
                    nc.scalar.activation(out=ep[pi][:, sub, 0:N // 2],
                                         in_=srow[:], func=Exp, bias=nmax7[:],
                                         accum_out=rs1[:])
                    rowsum = stat.tile([P, 1], F32, tag="rowsum")
                    nc.vector.tensor_add(rowsum[:], rs1[:], rs2[:])
                    rinv = stat.tile([P, 1], F32, tag="rinv")
                    nc.vector.reciprocal(out=rinv[:], in_=rowsum[:])
                    for t, vtsrc in ((0, VT_sb), (1, VthT_sb)):
                        vtmp = vtspool.tile([P, C], F32, tag="vtmp", bufs=3)
                        nc.vector.tensor_scalar(
                            out=vtmp[:], in0=vtsrc[:, blk, :], scalar1=rinv[:],
                            scalar2=float(2 ** VTS_SH), op0=Alu.mult,
                            op1=Alu.mult)
                        nc.scalar.copy(out=vh[(pi, t)][:, sub, :], in_=vtmp[:])
                        nc.gpsimd.tensor_sub(vl[(pi, t)][:, sub, :], vtmp[:],
                                             vh[(pi, t)][:, sub, :])
                    if prev is not None:
                        prev.emit(UNITS_PER_BLK)
                if prev is not None:
                    prev.emit(len(prev.units))
                prev = PEmitter(si, ep, vh, vl)
            prev.emit(len(prev.units))


        # =============== phase 2: channel attention ===============
        with ExitStack() as cctx:
            cpool = cctx.enter_context(tc.tile_pool(name="chan", bufs=1))
            cpsum = cctx.enter_context(tc.tile_pool(name="chan_psum", bufs=2, space="PSUM"))
            ctr = cctx.enter_context(tc.tile_pool(name="chan_tr", bufs=3))

            # sim_c = softmax_d(minE[c] - E[c, d]) (== the max-E softmax)
            emin = cpool.tile([P, CH], F32, tag="emin")
            esum = cpool.tile([P, CH], F32, tag="esum")
            einv = cpool.tile([P, CH], F32, tag="einv")
            simc = cpool.tile([P, CH, C], F32, tag="simc")
            for ch in range(CH):
                nc.vector.tensor_reduce(out=emin[:, ch:ch + 1], in_=E_sb[:, ch, :],
                                        axis=AX.X, op=Alu.min)
                nc.scalar.activation(out=simc[:, ch, :], in_=E_sb[:, ch, :], func=Exp,
                                     bias=emin[:, ch:ch + 1], scale=-1.0,
                                     accum_out=esum[:, ch:ch + 1])
                nc.vector.reciprocal(out=einv[:, ch:ch + 1], in_=esum[:, ch:ch + 1])
                nc.vector.tensor_scalar_mul(simc[:, ch, :], simc[:, ch, :],
                                            einv[:, ch:ch + 1])

            # transpose sim_c -> simcT [d, c] (bf16) via PE
            simcT = cpool.tile([P, CH, C], BF16, tag="simcT")
            for a in range(CH):
                for bnk in range(CH):
                    tps = cpsum.tile([P, P], F32, tag="tps")
                    nc.tensor.transpose(tps[:], simc[:, a, bnk * P:(bnk + 1) * P],
                                        identity[:])
                    nc.scalar.copy(out=simcT[:, bnk, a * P:(a + 1) * P], in_=tps[:])

            # x2att columns (my half): A = gamma2 * simc @ V + x_my
            for (vn, src_d, g2, out_d) in ((Vn_sb, xmy_d, g2_rgb, A_d),
                                           (Vthn_sb, xthmy_d, g2_th, Ath_d)):
                for ch in range(CH):
                    xr = ctr.tile([P, HALF], F32, tag="xres")
                    nc.sync.dma_start(out=xr[:],
                                      in_=src_d[ch * P:(ch + 1) * P, :])
                    ast = ctr.tile([P, HALF], F32, tag="astage")
                    for mc in range(NMC_H):
                        ps = cpsum.tile([P, MC], F32, tag="aps")
                        for dh in range(CH):
                            nc.tensor.matmul(ps[:],
                                             simcT[:, dh, ch * P:(ch + 1) * P],
                                             vn[:, dh, mc * MC:(mc + 1) * MC],
                                             start=(dh == 0), stop=(dh == CH - 1))
                        nc.vector.scalar_tensor_tensor(
                            out=ast[:, mc * MC:(mc + 1) * MC], in0=ps[:],
                            scalar=g2, in1=xr[:, mc * MC:(mc + 1) * MC],
                            op0=Alu.mult, op1=Alu.add)
                    nc.sync.dma_start(out=out_d[ch * P:(ch + 1) * P, :], in_=ast[:])

_PROGRAM_CACHE = {}
_LAST_IN_MAPS = None


def kernel(**inputs):
    global _LAST_IN_MAPS
    x = np.ascontiguousarray(inputs["x"], dtype=np.float32)        # [4, 256, 64, 64]
    x_th = np.ascontiguousarray(inputs["x_th"], dtype=np.float32)
    b = x.shape[0]
    gammas = tuple(float(np.asarray(inputs[k]).reshape(-1)[0])
                   for k in ("gamma_rgb", "gamma_th", "gamma2_rgb", "gamma2_th"))

    if gammas not in _PROGRAM_CACHE:
        _PROGRAM_CACHE[gammas] = build_program(*gammas)
    nc = _PROGRAM_CACHE[gammas]

    # packed const buffer [128, WCOLS]:
    #   [0:512] wkT, [512:1024] wkthT, [1024:1536] wvT, [1536:2048] wvthT
    #     (col = ci*256 + c_out; row p = c_in within half ci)
    #   [2048:2056] biases bk|bkth|bv|bvth (2 cols each, col h -> b[h*128+p])
    #   [2056:2568] bvb | bvthb broadcast rows (row p, col c -> b[c])
    wconst = np.zeros((P, WCOLS), np.float32)

    def pack_w(dst_off, W):
        wt = np.asarray(W, np.float32).T.reshape(CH, P, C).transpose(1, 0, 2)
        wconst[:, dst_off:dst_off + 512] = wt.reshape(P, 512)

    pack_w(0, inputs["Wk_rgb"])
    pack_w(512, inputs["Wk_th"])
    pack_w(1024, inputs["Wv_rgb"])
    pack_w(1536, inputs["Wv_th"])
    for t, key in enumerate(("bk_rgb", "bk_th", "bv_rgb", "bv_th")):
        bias = np.asarray(inputs[key], np.float32)
        wconst[:, 2048 + 2 * t: 2048 + 2 * t + 2] = bias.reshape(CH, P).T
    wconst[:, 2056:2056 + C] = np.asarray(inputs["bv_rgb"], np.float32)[None, :]
    wconst[:, 2056 + C:2056 + 2 * C] = np.asarray(inputs["bv_th"], np.float32)[None, :]

    in_maps = []
    for k in range(8):
        s, hh = divmod(k, 2)
        xs = x[s].reshape(C, N)
        xths = x_th[s].reshape(C, N)
        lo, hi = hh * HALF, (hh + 1) * HALF
        olo, ohi = (HALF, N) if hh == 0 else (0, HALF)
        # reorder: my half first (V^T blocks 0..15 = my sim rows)
        xv = np.concatenate([xs[:, lo:hi], xs[:, olo:ohi]], axis=1)
        xvth = np.concatenate([xths[:, lo:hi], xths[:, olo:ohi]], axis=1)
        m = {
            "x": xs,
            "xv": np.ascontiguousarray(xv),
            "xvth": np.ascontiguousarray(xvth),
            "x_my": np.ascontiguousarray(xs[:, lo:hi]),
            "xth_my": np.ascontiguousarray(xths[:, lo:hi]),
            "xth_my_r": np.ascontiguousarray(xths[:, lo:hi]),
            "wconst": wconst,
        }
        in_maps.append(m)

    _LAST_IN_MAPS = (nc, in_maps)
    res = run_bass_kernel_spmd(nc, in_maps, core_ids=list(range(8))).results

    g_rgb, g_th = gammas[0] * 2.0 ** -VTS_SH, gammas[1] * 2.0 ** -VTS_SH
    out = np.empty((b, C, N), np.float32)
    out_th = np.empty((b, C, N), np.float32)
    for s in range(b):
        k0, k1 = 2 * s, 2 * s + 1
        for (dst, a_key, p_key, gam) in ((out, "A_out", "P_out", g_rgb),
                                         (out_th, "Ath_out", "Pth_out", g_th)):
            acc = np.concatenate([res[k0][a_key], res[k1][a_key]], axis=1)
            acc = acc + gam * (res[k0][p_key].astype(np.float32).sum(axis=0)
                               + res[k1][p_key].astype(np.float32).sum(axis=0))
            dst[s] = acc

    w = int(np.sqrt(N))
    return out.reshape(b, C, w, w), out_th.reshape(b, C, w, w)


def timed_run(inputs=None):
    """Re-run the last compiled program with NTFF profiling; return exec ns."""
    if _LAST_IN_MAPS is None:
        if inputs is not None:
            kernel(**inputs)
        else:
            return None
    nc, in_maps = _LAST_IN_MAPS
    try:
        r = run_bass_kernel_spmd(nc, in_maps, core_ids=list(range(8)), trace=True)
        return r.exec_time_ns
    except Exception as e:  # profiling infra may be unavailable
        print(f"timed_run trace failed: {e}")
        return None

